# revision 4
# baseline (speedup 1.0000x reference)
"""Trainium2 Bass kernel for the spiking conv encoder (nn_Encoder_15410342658418).

Shapes (hardcoded): spike [8,2,128,128,32] -> out [8,32,64,64,32].
Data-parallel over batch N=8, one sample per NeuronCore.

t-synchronous per-core pipeline (one pass over t=0..31):
  * conv as im2col matmul, 2 matmuls per t (q halves) into persistent PSUM
    accumulators.  The CUBA current filter cur_t = sum_d 0.75^(t-d) z_d is
    folded INTO the PE accumulation: the host pre-scales rhs for step t by
    0.75^-t and the ACT evacuation applies scale 0.75^t -- so PSUM holds a
    weighted cumulative sum and ACT writes out exactly cur_t.  No scan pass.
  * ACT evacuates PSUM -> U[:, t+1, :] with the 0.75^t rescale.
  * DVE runs the LIF voltage recurrence as ONE fused custom op per step:
      u_t = select(u_{t-1} < 1, u_{t-1}, 0) * 0.9 + cur_t   (in place in U)
  * spike extraction + per-channel fractional delay mix, per 4-step group:
      out_t = (1-f)*[u_t >= 1] + f*[u_{t-1} >= 1]           (bf16)
    split across engines: sigma on ACT (Sign, cols < Q1) and DVE (is_ge TS
    2x, cols >= Q1); two affine passes at 4x bf16 on DVE; final add as a
    Pool tensor_tensor.  Host upcasts the bf16 result to fp32.
"""

import numpy as np

import concourse.bacc as bacc
import concourse.bass_utils as bass_utils
import concourse.tile as tile
from concourse import mybir

# ---- custom DVE op registration (runtime, self-contained) ----
from concourse.dve_spec import Spec, Src0, Src1, C0, select, lower, One, Zero
from concourse import dve_ops as _dve_ops
from concourse.dve_uop import DveOpSpec


def _register_op(name, spec, subdim=False):
    existing = {op.name: op for op in _dve_ops.OPS}
    if name in existing:
        return existing[name]
    shas = {}
    for ver in ("v3", "v4"):
        try:
            shas[ver] = DveOpSpec(name=name, uops=lower(spec, ver=ver)).sha(ver)
        except Exception:
            pass
    op = _dve_ops.DveOp(name, spec, subdim=subdim, uops_sha=shas)
    _dve_ops.OPS.append(op)
    _dve_ops._SUB_OPCODE_FOR_NAME[name] = (
        _dve_ops._CUSTOM_DVE_ROW_BASE + len(_dve_ops.OPS) - 1
    )
    return op


# u_t = select(u_{t-1} < 1, u_{t-1}, 0) * s0 + cur_t
LIF_STEP = _register_op(
    "LIF_STEP_ANT",
    Spec(
        body=select(Src0 < One, Src0, Zero) * C0 + Src1,
        reference=lambda in0, in1, s0, s1, imm2: (
            np.where(in0 < 1.0, in0, 0.0) * s0 + in1
        ).astype(np.float32),
    ),
)

N, C, H, W, T = 8, 2, 128, 128, 32
CH = 32
Hp, Wp = 64, 64
CUR_DECAY = 0.25
VOLT_DECAY = 0.1
LEAK = 1.0 - VOLT_DECAY  # 0.9
DECAY = 1.0 - CUR_DECAY  # 0.75
YB = 4
NYG = Hp // YB  # 16 y-groups
K = 72  # contraction rows (kx, c, ky*4+yb)
Q = NYG * Wp  # 1024 state columns
QH = Q // 2
TG = 4  # t-steps per output group
NG = T // TG  # 8 output groups
Q1 = 704  # sigma split: cols < Q1 on ACT (Sign, +-1), >= Q1 on DVE ({0,1})

_COMPILED = None


def _build_program():
    nc = bacc.Bacc("TRN2", target_bir_lowering=False, debug=False, num_devices=N)
    f32 = mybir.dt.float32
    bf16 = mybir.dt.bfloat16

    x_d = nc.dram_tensor("x", [T, K, Q], f32, kind="ExternalInput")
    wblk_d = nc.dram_tensor("wblk", [K, 128], f32, kind="ExternalInput")
    coef_d = nc.dram_tensor("coef", [128, 4], f32, kind="ExternalInput")
    out_d = nc.dram_tensor("out", [NG, 128, TG * Q], bf16, kind="ExternalOutput")

    from contextlib import ExitStack

    with tile.TileContext(nc) as tc, ExitStack() as ctx:
        _kernel_body(ctx, tc, x_d.ap(), wblk_d.ap(), coef_d.ap(), out_d.ap())
    nc.compile()
    return nc


def _kernel_body(ctx, tc, x, wblk, coef, out):
    nc = tc.nc
    f32 = mybir.dt.float32
    bf16 = mybir.dt.bfloat16
    Act = mybir.ActivationFunctionType
    Alu = mybir.AluOpType

    consts = ctx.enter_context(tc.tile_pool(name="consts", bufs=1))
    xpool = ctx.enter_context(tc.tile_pool(name="xpool", bufs=3))
    upool = ctx.enter_context(tc.tile_pool(name="upool", bufs=1))
    sgpool = ctx.enter_context(tc.tile_pool(name="sgpool", bufs=2))
    mqpool = ctx.enter_context(tc.tile_pool(name="mqpool", bufs=1))
    opool = ctx.enter_context(tc.tile_pool(name="opool", bufs=2))
    psump = ctx.enter_context(tc.tile_pool(name="psump", bufs=1, space="PSUM"))

    wblk_t = consts.tile([K, 128], f32)
    nc.sync.dma_start(out=wblk_t, in_=wblk)
    coef_t = consts.tile([128, 4], f32)
    nc.sync.dma_start(out=coef_t, in_=coef)
    f_ap = coef_t[:, 0:1]  # f
    omf_ap = coef_t[:, 1:2]  # 1 - f
    a_ap = coef_t[:, 2:3]  # -(1-f)/2
    b_ap = coef_t[:, 3:4]  # -f/2

    # U[:, s, :]: s=0 zeros (u_{-1}); s=t+1 holds cur_t, overwritten by u_t.
    U = upool.tile([128, (T + 1) * Q], f32)
    U3 = U.rearrange("p (s q) -> p s q", q=Q)
    nc.vector.memset(U3[:, 0, :], 0.0)

    # sigma of u_{-1}: Sign encoding -1 (cols < Q1), {0,1} encoding 0.
    sg_init = consts.tile([128, Q], bf16)
    nc.vector.memset(sg_init[:, 0:Q1], 1.0)
    nc.vector.memset(sg_init[:, Q1:Q], 0.0)

    psA = psump.tile([128, QH], f32, name="psA", tag="psA")
    psB = psump.tile([128, QH], f32, name="psB", tag="psB")

    xt = [None] * T
    sgbufs = [None] * NG

    def load_x(t):
        xt[t] = xpool.tile([K, Q], f32, tag="xt", name=f"x{t}")
        nc.sync.dma_start(out=xt[t], in_=x[t])

    def sigma_group(g):
        """Extract spike indicators for u_t, t in [4g, 4g+4) -> sgbuf [128, TG*Q]."""
        sg = sgpool.tile([128, TG * Q], bf16, tag="sg", name=f"sg{g}")
        sg3 = sg.rearrange("p (s q) -> p s q", q=Q)
        usl = U3[:, TG * g + 1 : TG * g + 1 + TG, :]
        # ACT part: sigma' = Sign(1 - u) in {-1, 0, 1} (+1 = no spike)
        nc.scalar.activation(
            out=sg3[:, :, 0:Q1], in_=usl[:, :, 0:Q1], func=Act.Sign,
            bias=1.0, scale=-1.0,
        )
        # DVE part: s = (u >= 1) in {0, 1}, 2x mode
        nc.vector.tensor_scalar(
            out=sg3[:, :, Q1:Q], in0=usl[:, :, Q1:Q], scalar1=1.0, scalar2=None,
            op0=Alu.is_ge,
        )
        sgbufs[g] = sg
        return sg

    def mix_group(g, mq):
        """m = coef*enc(u_t) [+1/4], q2 = coef*enc(u_{t-1}) [+1/4]; out = m+q2."""
        sg = sgbufs[g]
        sg3 = sg.rearrange("p (s q) -> p s q", q=Q)
        m3 = mq.rearrange("p (h s q) -> p h s q", h=2, q=Q)[:, 0]
        q3 = mq.rearrange("p (h s q) -> p h s q", h=2, q=Q)[:, 1]
        # m: from sg(g) slots 0..3
        nc.vector.tensor_scalar(
            out=m3[:, :, 0:Q1], in0=sg3[:, :, 0:Q1],
            scalar1=a_ap, scalar2=0.25, op0=Alu.mult, op1=Alu.add,
        )
        nc.vector.tensor_scalar(
            out=m3[:, :, Q1:Q], in0=sg3[:, :, Q1:Q],
            scalar1=omf_ap, scalar2=None, op0=Alu.mult,
        )
        # q2 slot 0: sigma(u_{4g-1}) from sg(g-1) slot 3 (or sg_init)
        prev = sg_init if g == 0 else sgbufs[g - 1].rearrange(
            "p (s q) -> p s q", q=Q
        )[:, 3]
        nc.vector.tensor_scalar(
            out=q3[:, 0, 0:Q1], in0=prev[:, 0:Q1],
            scalar1=b_ap, scalar2=0.25, op0=Alu.mult, op1=Alu.add,
        )
        nc.vector.tensor_scalar(
            out=q3[:, 0, Q1:Q], in0=prev[:, Q1:Q],
            scalar1=f_ap, scalar2=None, op0=Alu.mult,
        )
        # q2 slots 1..3: sigma(u_t), t = 4g..4g+2 from sg(g) slots 0..2
        nc.vector.tensor_scalar(
            out=q3[:, 1:TG, 0:Q1], in0=sg3[:, 0 : TG - 1, 0:Q1],
            scalar1=b_ap, scalar2=0.25, op0=Alu.mult, op1=Alu.add,
        )
        nc.vector.tensor_scalar(
            out=q3[:, 1:TG, Q1:Q], in0=sg3[:, 0 : TG - 1, Q1:Q],
            scalar1=f_ap, scalar2=None, op0=Alu.mult,
        )

    for t in range(2):
        load_x(t)

    mq = mqpool.tile([128, 2 * TG * Q], bf16)

    for t in range(T):
        if t + 2 < T:
            load_x(t + 2)
        nc.tensor.matmul(
            psA, lhsT=wblk_t, rhs=xt[t][:, 0:QH], start=(t == 0), stop=(t == T - 1)
        )
        nc.tensor.matmul(
            psB, lhsT=wblk_t, rhs=xt[t][:, QH:Q], start=(t == 0), stop=(t == T - 1)
        )
        sc = float(DECAY**t)
        nc.scalar.activation(out=U3[:, t + 1, 0:QH], in_=psA, func=Act.Copy, scale=sc)
        nc.scalar.activation(out=U3[:, t + 1, QH:Q], in_=psB, func=Act.Copy, scale=sc)
        # u_t = select(u_{t-1} < 1, u_{t-1}, 0) * 0.9 + cur_t   (in place)
        nc.vector._custom_dve(
            LIF_STEP,
            out=U3[:, t + 1, :],
            in0=U3[:, t, :],
            in1=U3[:, t + 1, :],
            s0=LEAK,
        )
        if t % TG == TG - 1:
            g = t // TG
            sigma_group(g)
            mix_group(g, mq)
            ob = opool.tile([128, TG * Q], bf16, tag="ob", name=f"ob{g}")
            nc.gpsimd.tensor_tensor(
                out=ob, in0=mq[:, 0 : TG * Q], in1=mq[:, TG * Q : 2 * TG * Q],
                op=Alu.add,
            )
            nc.sync.dma_start(out=out[g], in_=ob)


def _host_prep(spike, weight_v, weight_g, delay):
    spike = np.asarray(spike, dtype=np.float32)
    weight_v = np.asarray(weight_v, dtype=np.float32)
    weight_g = np.asarray(weight_g, dtype=np.float32)
    delay = np.asarray(delay, dtype=np.float32)

    vnorm = np.sqrt((weight_v * weight_v).sum(axis=(1, 2, 3), keepdims=True))
    wn = (weight_g[:, None, None, None] * weight_v / vnorm).astype(np.float32)

    # lhsT [72, 128]: row kx*24 + c*12 + ky*4 + yb -> col yb*32 + ch
    wblk = np.zeros((K, 128), dtype=np.float32)
    for yb in range(YB):
        for kx in range(3):
            for c in range(C):
                for ky in range(3):
                    row = kx * 24 + c * 12 + ky * 4 + yb
                    wblk[row, yb * 32 : (yb + 1) * 32] = wn[:, c, ky, kx]

    f = delay.astype(np.float32)
    coef = np.zeros((128, 4), dtype=np.float32)
    for yb in range(YB):
        s = slice(yb * 32, (yb + 1) * 32)
        coef[s, 0] = f
        coef[s, 1] = 1.0 - f
        coef[s, 2] = -(1.0 - f) / 2.0
        coef[s, 3] = -f / 2.0

    # im2col, t-major, pre-scaled by 0.75^-t: xrep[n, t, krow, yg*64+x]
    xpad = np.pad(spike, ((0, 0), (0, 0), (1, 0), (1, 0), (0, 0)))
    xrep = np.empty((N, T, K, Q), dtype=np.float32)
    yg8 = 8 * np.arange(NYG)
    for kx in range(3):
        for ky in range(3):
            for yb in range(4):
                rows = 2 * yb + ky + yg8
                # [n, c, yg, x, t]
                blk = xpad[:, :, rows, kx : kx + 2 * Wp : 2, :]
                for c in range(C):
                    row = kx * 24 + c * 12 + ky * 4 + yb
                    xrep[:, :, row, :] = (
                        blk[:, c].transpose(0, 3, 1, 2).reshape(N, T, Q)
                    )
    upsc = (np.float32(DECAY) ** (-np.arange(T, dtype=np.float32))).astype(np.float32)
    xrep *= upsc[None, :, None, None]
    return xrep, wblk, coef


def _host_post(outs):
    # per-core out [NG, 128, TG*Q] bf16 -> [CH, 64, 64, T] fp32
    full = np.empty((N, CH, Hp, Wp, T), dtype=np.float32)
    for n, o in enumerate(outs):
        a = np.asarray(o).astype(np.float32)
        # [tg, (yb,ch), tt, yg, x] -> [ch, yg, yb, x, tg, tt]
        a = a.reshape(NG, YB, CH, TG, NYG, Wp).transpose(2, 4, 1, 5, 0, 3)
        full[n] = a.reshape(CH, Hp, Wp, T)
    return full


def kernel(spike, weight_v, weight_g, delay):
    global _COMPILED
    if _COMPILED is None:
        _COMPILED = _build_program()
    nc = _COMPILED

    xrep, wblk, coef = _host_prep(spike, weight_v, weight_g, delay)
    in_maps = [
        {"x": np.ascontiguousarray(xrep[n]), "wblk": wblk, "coef": coef}
        for n in range(N)
    ]
    res = bass_utils.run_bass_kernel_spmd(nc, in_maps, core_ids=list(range(N)))
    return _host_post([r["out"] for r in res.results])


# revision 5
# speedup vs baseline: 1.4085x; 1.4085x over previous
"""Trainium2 Bass kernel for the spiking conv encoder (nn_Encoder_15410342658418).

Shapes (hardcoded): spike [8,2,128,128,32] -> out [8,32,64,64,32].
Data-parallel over batch N=8, one sample per NeuronCore.

t-synchronous per-core pipeline (one pass over t=0..31):
  * conv as im2col matmul, 2 matmuls per t (q halves) into persistent PSUM
    accumulators.  The CUBA current filter cur_t = sum_d 0.75^(t-d) z_d is
    folded INTO the PE accumulation: the host pre-scales rhs for step t by
    0.75^-t and the ACT evacuation applies scale 0.75^t -- so PSUM holds a
    weighted cumulative sum and ACT writes out exactly cur_t.  No scan pass.
  * ACT evacuates PSUM -> U[:, t+1, :] with the 0.75^t rescale.
  * DVE runs the LIF voltage recurrence as ONE fused custom op per step:
      u_t = select(u_{t-1} < 1, u_{t-1}, 0) * 0.9 + cur_t   (in place in U)
  * spike extraction + per-channel fractional delay mix, per 4-step group:
      out_t = (1-f)*[u_t >= 1] + f*[u_{t-1} >= 1]           (bf16)
    split across engines: sigma on ACT (Sign, cols < Q1) and DVE (is_ge TS
    2x, cols >= Q1); two affine passes at 4x bf16 on DVE; final add as a
    Pool tensor_tensor.  Host upcasts the bf16 result to fp32.
"""

import numpy as np

import concourse.bacc as bacc
import concourse.bass_utils as bass_utils
import concourse.tile as tile
from concourse import mybir

# ---- custom DVE op registration (runtime, self-contained) ----
from concourse.dve_spec import Spec, Src0, Src1, C0, C1, select, lower, One, Zero
from concourse import dve_ops as _dve_ops
from concourse.dve_uop import DveOpSpec


def _register_op(name, spec, subdim=False):
    existing = {op.name: op for op in _dve_ops.OPS}
    if name in existing:
        return existing[name]
    shas = {}
    for ver in ("v3", "v4"):
        try:
            shas[ver] = DveOpSpec(name=name, uops=lower(spec, ver=ver)).sha(ver)
        except Exception:
            pass
    op = _dve_ops.DveOp(name, spec, subdim=subdim, uops_sha=shas)
    _dve_ops.OPS.append(op)
    _dve_ops._SUB_OPCODE_FOR_NAME[name] = (
        _dve_ops._CUSTOM_DVE_ROW_BASE + len(_dve_ops.OPS) - 1
    )
    return op


# u_t = select(u_{t-1} < 1, u_{t-1}, 0) * s0 + cur_t
LIF_STEP = _register_op(
    "LIF_STEP_ANT",
    Spec(
        body=select(Src0 < One, Src0, Zero) * C0 + Src1,
        reference=lambda in0, in1, s0, s1, imm2: (
            np.where(in0 < 1.0, in0, 0.0) * s0 + in1
        ).astype(np.float32),
    ),
)

# out_t = select(u_t>=1, c1, 0) + select(u_{t-1}>=1, c0, 0)  (delay mix)
DELAY_MIX = _register_op(
    "DELAY_MIX_ANT",
    Spec(
        body=select(Src1 >= One, C1, Zero) + select(Src0 >= One, C0, Zero),
        reference=lambda in0, in1, s0, s1, imm2: (
            np.where(in1 >= 1.0, s1, 0.0) + np.where(in0 >= 1.0, s0, 0.0)
        ).astype(np.float32),
    ),
)

N, C, H, W, T = 8, 2, 128, 128, 32
CH = 32
Hp, Wp = 64, 64
CUR_DECAY = 0.25
VOLT_DECAY = 0.1
LEAK = 1.0 - VOLT_DECAY  # 0.9
DECAY = 1.0 - CUR_DECAY  # 0.75
YB = 4
NYG = Hp // YB  # 16 y-groups
K = 72  # contraction rows (kx, c, ky*4+yb)
Q = NYG * Wp  # 1024 state columns
QH = Q // 2
TG = 4  # t-steps per output group
NG = T // TG  # 8 output groups
Q1 = 704  # sigma split: cols < Q1 on ACT (Sign, +-1), >= Q1 on DVE ({0,1})

_COMPILED = None


def _build_program():
    nc = bacc.Bacc("TRN2", target_bir_lowering=False, debug=False, num_devices=N)
    f32 = mybir.dt.float32
    bf16 = mybir.dt.bfloat16

    x_d = nc.dram_tensor("x", [T, K, Q], f32, kind="ExternalInput")
    wblk_d = nc.dram_tensor("wblk", [K, 128], f32, kind="ExternalInput")
    coef_d = nc.dram_tensor("coef", [128, 4], f32, kind="ExternalInput")
    out_d = nc.dram_tensor("out", [NG, 128, TG * Q], bf16, kind="ExternalOutput")

    from contextlib import ExitStack

    with tile.TileContext(nc) as tc, ExitStack() as ctx:
        _kernel_body(ctx, tc, x_d.ap(), wblk_d.ap(), coef_d.ap(), out_d.ap())
    nc.compile()
    return nc


def _kernel_body(ctx, tc, x, wblk, coef, out):
    nc = tc.nc
    f32 = mybir.dt.float32
    bf16 = mybir.dt.bfloat16
    Act = mybir.ActivationFunctionType
    Alu = mybir.AluOpType

    consts = ctx.enter_context(tc.tile_pool(name="consts", bufs=1))
    xpool = ctx.enter_context(tc.tile_pool(name="xpool", bufs=3))
    upool = ctx.enter_context(tc.tile_pool(name="upool", bufs=1))
    opool = ctx.enter_context(tc.tile_pool(name="opool", bufs=2))
    psump = ctx.enter_context(tc.tile_pool(name="psump", bufs=1, space="PSUM"))

    wblk_t = consts.tile([K, 128], f32)
    nc.sync.dma_start(out=wblk_t, in_=wblk)
    coef_t = consts.tile([128, 4], f32)
    nc.sync.dma_start(out=coef_t, in_=coef)
    f_ap = coef_t[:, 0:1]  # f
    omf_ap = coef_t[:, 1:2]  # 1 - f
    a_ap = coef_t[:, 2:3]  # -(1-f)/2
    b_ap = coef_t[:, 3:4]  # -f/2

    # U[:, s, :]: s=0 zeros (u_{-1}); s=t+1 holds cur_t, overwritten by u_t.
    U = upool.tile([128, (T + 1) * Q], f32)
    U3 = U.rearrange("p (s q) -> p s q", q=Q)
    nc.vector.memset(U3[:, 0, :], 0.0)

    psA = psump.tile([128, QH], f32, name="psA", tag="psA")
    psB = psump.tile([128, QH], f32, name="psB", tag="psB")

    xt = [None] * T

    def load_x(t):
        xt[t] = xpool.tile([K, Q], f32, tag="xt", name=f"x{t}")
        nc.sync.dma_start(out=xt[t], in_=x[t])

    for t in range(2):
        load_x(t)

    for t in range(T):
        if t + 2 < T:
            load_x(t + 2)
        nc.tensor.matmul(
            psA, lhsT=wblk_t, rhs=xt[t][:, 0:QH], start=(t == 0), stop=(t == T - 1)
        )
        nc.tensor.matmul(
            psB, lhsT=wblk_t, rhs=xt[t][:, QH:Q], start=(t == 0), stop=(t == T - 1)
        )
        sc = float(DECAY**t)
        nc.scalar.activation(out=U3[:, t + 1, 0:QH], in_=psA, func=Act.Copy, scale=sc)
        nc.scalar.activation(out=U3[:, t + 1, QH:Q], in_=psB, func=Act.Copy, scale=sc)
        # u_t = select(u_{t-1} < 1, u_{t-1}, 0) * 0.9 + cur_t   (in place)
        nc.vector._custom_dve(
            LIF_STEP,
            out=U3[:, t + 1, :],
            in0=U3[:, t, :],
            in1=U3[:, t + 1, :],
            s0=LEAK,
        )
        if t % TG == TG - 1:
            g = t // TG
            ob = opool.tile([128, TG * Q], bf16, tag="ob", name=f"ob{g}")
            nc.vector._custom_dve(
                DELAY_MIX,
                out=ob,
                in0=U[:, g * TG * Q : (g + 1) * TG * Q],
                in1=U[:, (g * TG + 1) * Q : ((g + 1) * TG + 1) * Q],
                s0=f_ap,
                s1=omf_ap,
            )
            nc.gpsimd.dma_start(out=out[g], in_=ob)


def _host_prep(spike, weight_v, weight_g, delay):
    spike = np.asarray(spike, dtype=np.float32)
    weight_v = np.asarray(weight_v, dtype=np.float32)
    weight_g = np.asarray(weight_g, dtype=np.float32)
    delay = np.asarray(delay, dtype=np.float32)

    vnorm = np.sqrt((weight_v * weight_v).sum(axis=(1, 2, 3), keepdims=True))
    wn = (weight_g[:, None, None, None] * weight_v / vnorm).astype(np.float32)

    # lhsT [72, 128]: row kx*24 + c*12 + ky*4 + yb -> col yb*32 + ch
    wblk = np.zeros((K, 128), dtype=np.float32)
    for yb in range(YB):
        for kx in range(3):
            for c in range(C):
                for ky in range(3):
                    row = kx * 24 + c * 12 + ky * 4 + yb
                    wblk[row, yb * 32 : (yb + 1) * 32] = wn[:, c, ky, kx]

    f = delay.astype(np.float32)
    coef = np.zeros((128, 4), dtype=np.float32)
    for yb in range(YB):
        s = slice(yb * 32, (yb + 1) * 32)
        coef[s, 0] = f
        coef[s, 1] = 1.0 - f
        coef[s, 2] = -(1.0 - f) / 2.0
        coef[s, 3] = -f / 2.0

    # im2col, t-major, pre-scaled by 0.75^-t: xrep[n, t, krow, yg*64+x]
    xpad = np.pad(spike, ((0, 0), (0, 0), (1, 0), (1, 0), (0, 0)))
    xrep = np.empty((N, T, K, Q), dtype=np.float32)
    yg8 = 8 * np.arange(NYG)
    for kx in range(3):
        for ky in range(3):
            for yb in range(4):
                rows = 2 * yb + ky + yg8
                # [n, c, yg, x, t]
                blk = xpad[:, :, rows, kx : kx + 2 * Wp : 2, :]
                for c in range(C):
                    row = kx * 24 + c * 12 + ky * 4 + yb
                    xrep[:, :, row, :] = (
                        blk[:, c].transpose(0, 3, 1, 2).reshape(N, T, Q)
                    )
    upsc = (np.float32(DECAY) ** (-np.arange(T, dtype=np.float32))).astype(np.float32)
    xrep *= upsc[None, :, None, None]
    return xrep, wblk, coef


def _host_post(outs):
    # per-core out [NG, 128, TG*Q] bf16 -> [CH, 64, 64, T] fp32
    full = np.empty((N, CH, Hp, Wp, T), dtype=np.float32)
    for n, o in enumerate(outs):
        a = np.asarray(o).astype(np.float32)
        # [tg, (yb,ch), tt, yg, x] -> [ch, yg, yb, x, tg, tt]
        a = a.reshape(NG, YB, CH, TG, NYG, Wp).transpose(2, 4, 1, 5, 0, 3)
        full[n] = a.reshape(CH, Hp, Wp, T)
    return full


def kernel(spike, weight_v, weight_g, delay):
    global _COMPILED
    if _COMPILED is None:
        _COMPILED = _build_program()
    nc = _COMPILED

    xrep, wblk, coef = _host_prep(spike, weight_v, weight_g, delay)
    in_maps = [
        {"x": np.ascontiguousarray(xrep[n]), "wblk": wblk, "coef": coef}
        for n in range(N)
    ]
    res = bass_utils.run_bass_kernel_spmd(nc, in_maps, core_ids=list(range(N)))
    return _host_post([r["out"] for r in res.results])


# revision 8
# speedup vs baseline: 1.4989x; 1.0642x over previous
"""Trainium2 Bass kernel for the spiking conv encoder (nn_Encoder_15410342658418).

Shapes (hardcoded): spike [8,2,128,128,32] -> out [8,32,64,64,32].
Data-parallel over batch N=8, one sample per NeuronCore.

t-synchronous per-core pipeline (one pass over t=0..31):
  * conv as im2col matmul, 2 matmuls per t (q halves) into persistent PSUM
    accumulators.  The CUBA current filter cur_t = sum_d 0.75^(t-d) z_d is
    folded INTO the PE accumulation: the host pre-scales rhs for step t by
    0.75^-t and the ACT evacuation applies scale 0.75^t -- so PSUM holds a
    weighted cumulative sum and ACT writes out exactly cur_t.  No scan pass.
  * ACT evacuates PSUM -> U[:, t+1, :] with the 0.75^t rescale.
  * DVE runs the LIF voltage recurrence as ONE fused custom op per step:
      u_t = select(u_{t-1} < 1, u_{t-1}, 0) * 0.9 + cur_t   (in place in U)
  * spike extraction + per-channel fractional delay mix, per 4-step group:
      out_t = (1-f)*[u_t >= 1] + f*[u_{t-1} >= 1]           (bf16)
    split across engines: sigma on ACT (Sign, cols < Q1) and DVE (is_ge TS
    2x, cols >= Q1); two affine passes at 4x bf16 on DVE; final add as a
    Pool tensor_tensor.  Host upcasts the bf16 result to fp32.
"""

import numpy as np

import concourse.bacc as bacc
import concourse.bass_utils as bass_utils
import concourse.tile as tile
from concourse import mybir

# ---- custom DVE op registration (runtime, self-contained) ----
from concourse.dve_spec import Spec, Src0, Src1, C0, C1, select, lower, One, Zero
from concourse import dve_ops as _dve_ops
from concourse.dve_uop import DveOpSpec


def _register_op(name, spec, subdim=False):
    existing = {op.name: op for op in _dve_ops.OPS}
    if name in existing:
        return existing[name]
    shas = {}
    for ver in ("v3", "v4"):
        try:
            shas[ver] = DveOpSpec(name=name, uops=lower(spec, ver=ver)).sha(ver)
        except Exception:
            pass
    op = _dve_ops.DveOp(name, spec, subdim=subdim, uops_sha=shas)
    _dve_ops.OPS.append(op)
    _dve_ops._SUB_OPCODE_FOR_NAME[name] = (
        _dve_ops._CUSTOM_DVE_ROW_BASE + len(_dve_ops.OPS) - 1
    )
    return op


# u_t = select(u_{t-1} < 1, u_{t-1}, 0) * s0 + cur_t
LIF_STEP = _register_op(
    "LIF_STEP_ANT",
    Spec(
        body=select(Src0 < One, Src0, Zero) * C0 + Src1,
        reference=lambda in0, in1, s0, s1, imm2: (
            np.where(in0 < 1.0, in0, 0.0) * s0 + in1
        ).astype(np.float32),
    ),
)

# out_t = select(u_t>=1, 1-c0, 0) + select(u_{t-1}>=1, c0, 0)  (delay mix)
DELAY_MIX = _register_op(
    "DELAY_MIX2_ANT",
    Spec(
        body=select(Src1 >= One, One - C0, Zero) + select(Src0 >= One, C0, Zero),
        reference=lambda in0, in1, s0, s1, imm2: (
            np.where(in1 >= 1.0, 1.0 - s0, 0.0) + np.where(in0 >= 1.0, s0, 0.0)
        ).astype(np.float32),
    ),
)

# out_t = select(u_t>=1, 1-c0, 0) + select(u_{t-1}>=1, c0, 0)  (delay mix)
DELAY_MIX = _register_op(
    "DELAY_MIX2_ANT",
    Spec(
        body=select(Src1 >= One, One - C0, Zero) + select(Src0 >= One, C0, Zero),
        reference=lambda in0, in1, s0, s1, imm2: (
            np.where(in1 >= 1.0, 1.0 - s0, 0.0) + np.where(in0 >= 1.0, s0, 0.0)
        ).astype(np.float32),
    ),
)

N, C, H, W, T = 8, 2, 128, 128, 32
CH = 32
Hp, Wp = 64, 64
CUR_DECAY = 0.25
VOLT_DECAY = 0.1
LEAK = 1.0 - VOLT_DECAY  # 0.9
DECAY = 1.0 - CUR_DECAY  # 0.75
YB = 4
NYG = Hp // YB  # 16 y-groups
K = 72  # contraction rows (kx, c, ky*4+yb)
Q = NYG * Wp  # 1024 state columns
QH = Q // 2
TG = 4  # t-steps per output group
NG = T // TG  # 8 output groups
QC = 256  # cols [0,QC): custom DVE mix; [QC,Q): ACT sigma + DVE affine + Pool TT
QS = Q - QC

_COMPILED = None


def _build_program():
    nc = bacc.Bacc("TRN2", target_bir_lowering=False, debug=False, num_devices=N)
    f32 = mybir.dt.float32
    bf16 = mybir.dt.bfloat16

    x_d = nc.dram_tensor("x", [T, K, Q], f32, kind="ExternalInput")
    wblk_d = nc.dram_tensor("wblk", [K, 128], f32, kind="ExternalInput")
    coef_d = nc.dram_tensor("coef", [128, 6], f32, kind="ExternalInput")
    out_d = nc.dram_tensor("out", [NG, 128, TG * Q], bf16, kind="ExternalOutput")

    from contextlib import ExitStack

    with tile.TileContext(nc) as tc, ExitStack() as ctx:
        _kernel_body(ctx, tc, x_d.ap(), wblk_d.ap(), coef_d.ap(), out_d.ap())
    nc.compile()
    return nc


def _kernel_body(ctx, tc, x, wblk, coef, out):
    nc = tc.nc
    f32 = mybir.dt.float32
    bf16 = mybir.dt.bfloat16
    Act = mybir.ActivationFunctionType
    Alu = mybir.AluOpType

    consts = ctx.enter_context(tc.tile_pool(name="consts", bufs=1))
    xpool = ctx.enter_context(tc.tile_pool(name="xpool", bufs=3))
    upool = ctx.enter_context(tc.tile_pool(name="upool", bufs=1))
    opool = ctx.enter_context(tc.tile_pool(name="opool", bufs=3))
    sgpool = ctx.enter_context(tc.tile_pool(name="sgpool", bufs=2))
    mpool = ctx.enter_context(tc.tile_pool(name="mpool", bufs=2))
    qpool = ctx.enter_context(tc.tile_pool(name="qpool", bufs=2))
    ospool = ctx.enter_context(tc.tile_pool(name="ospool", bufs=2))
    psump = ctx.enter_context(tc.tile_pool(name="psump", bufs=1, space="PSUM"))

    wblk_t = consts.tile([K, 128], f32)
    nc.sync.dma_start(out=wblk_t, in_=wblk)
    coef_t = consts.tile([128, 6], f32)
    nc.sync.dma_start(out=coef_t, in_=coef)
    f_ap = coef_t[:, 0:1]  # f
    omf_ap = coef_t[:, 1:2]  # 1 - f
    f_ap = coef_t[:, 0:1]  # f
    a_ap = coef_t[:, 2:3]  # -(1-f)/2
    b_ap = coef_t[:, 3:4]  # -f/2
    a2_ap = coef_t[:, 4:5]  # (1-f)/2
    b2_ap = coef_t[:, 5:6]  # f/2

    # U[:, s, :]: s=0 zeros (u_{-1}); s=t+1 holds cur_t, overwritten by u_t.
    U = upool.tile([128, (T + 1) * Q], f32)
    U3 = U.rearrange("p (s q) -> p s q", q=Q)
    nc.vector.memset(U3[:, 0, :], 0.0)

    sg_init = consts.tile([128, QS], bf16)
    nc.vector.memset(sg_init, 1.0)

    psA = psump.tile([128, QH], f32, name="psA", tag="psA")
    psB = psump.tile([128, QH], f32, name="psB", tag="psB")

    xt = [None] * T
    sgbufs = [None] * NG

    def load_x(t):
        xt[t] = xpool.tile([K, Q], f32, tag="xt", name=f"x{t}")
        nc.sync.dma_start(out=xt[t], in_=x[t])

    for t in range(2):
        load_x(t)

    for t in range(T):
        if t + 2 < T:
            load_x(t + 2)
        nc.tensor.matmul(
            psA, lhsT=wblk_t, rhs=xt[t][:, 0:QH], start=(t == 0), stop=(t == T - 1)
        )
        nc.tensor.matmul(
            psB, lhsT=wblk_t, rhs=xt[t][:, QH:Q], start=(t == 0), stop=(t == T - 1)
        )
        sc = float(DECAY**t)
        nc.scalar.activation(out=U3[:, t + 1, 0:QH], in_=psA, func=Act.Copy, scale=sc)
        nc.scalar.activation(out=U3[:, t + 1, QH:Q], in_=psB, func=Act.Copy, scale=sc)
        # u_t = select(u_{t-1} < 1, u_{t-1}, 0) * 0.9 + cur_t   (in place)
        nc.vector._custom_dve(
            LIF_STEP,
            out=U3[:, t + 1, :],
            in0=U3[:, t, :],
            in1=U3[:, t + 1, :],
            s0=LEAK,
        )
        if t % TG == TG - 1:
            g = t // TG
            og = out[g].rearrange("p (s q) -> p s q", q=Q)
            # custom DVE mix on cols [0, QC)
            obc = opool.tile([128, TG * QC], bf16, tag="ob", name=f"ob{g}")
            nc.vector._custom_dve(
                DELAY_MIX,
                out=obc.rearrange("p (s q) -> p s q", q=QC),
                in0=U3[:, TG * g : TG * g + TG, 0:QC],
                in1=U3[:, TG * g + 1 : TG * g + 1 + TG, 0:QC],
                s0=f_ap,
            )
            nc.gpsimd.dma_start(
                out=og[:, :, 0:QC],
                in_=obc.rearrange("p (s q) -> p s q", q=QC),
            )
            # hybrid path on cols [QC, Q): ACT sigma' = Sign(1-u) (+1 no spike)
            sg = sgpool.tile([128, TG * QS], bf16, tag="sg", name=f"sg{g}")
            sg3 = sg.rearrange("p (s q) -> p s q", q=QS)
            nc.scalar.activation(
                out=sg3,
                in_=U3[:, TG * g + 1 : TG * g + 1 + TG, QC:Q],
                func=Act.Sign,
                bias=1.0,
                scale=-1.0,
            )
            sgbufs[g] = sg
            # m = a*sigma' + (1-f)/2 ; q2 = b*sigma'_{t-1} + f/2   (DVE, 4x bf16)
            mb = mpool.tile([128, TG * QS], bf16, tag="mb", name=f"mb{g}")
            nc.vector.tensor_scalar(
                out=mb, in0=sg, scalar1=a_ap, scalar2=a2_ap,
                op0=Alu.mult, op1=Alu.add,
            )
            qb = qpool.tile([128, TG * QS], bf16, tag="qb", name=f"qb{g}")
            prev = sg_init if g == 0 else sgbufs[g - 1][:, 3 * QS : 4 * QS]
            nc.vector.tensor_scalar(
                out=qb[:, 0:QS], in0=prev, scalar1=b_ap, scalar2=b2_ap,
                op0=Alu.mult, op1=Alu.add,
            )
            nc.vector.tensor_scalar(
                out=qb[:, QS : TG * QS], in0=sg[:, 0 : 3 * QS],
                scalar1=b_ap, scalar2=b2_ap, op0=Alu.mult, op1=Alu.add,
            )
            obs = ospool.tile([128, TG * QS], bf16, tag="os", name=f"os{g}")
            nc.gpsimd.tensor_tensor(out=obs, in0=mb, in1=qb, op=Alu.add)
            nc.gpsimd.dma_start(
                out=og[:, :, QC:Q],
                in_=obs.rearrange("p (s q) -> p s q", q=QS),
            )


def _host_prep(spike, weight_v, weight_g, delay):
    spike = np.asarray(spike, dtype=np.float32)
    weight_v = np.asarray(weight_v, dtype=np.float32)
    weight_g = np.asarray(weight_g, dtype=np.float32)
    delay = np.asarray(delay, dtype=np.float32)

    vnorm = np.sqrt((weight_v * weight_v).sum(axis=(1, 2, 3), keepdims=True))
    wn = (weight_g[:, None, None, None] * weight_v / vnorm).astype(np.float32)

    # lhsT [72, 128]: row kx*24 + c*12 + ky*4 + yb -> col yb*32 + ch
    wblk = np.zeros((K, 128), dtype=np.float32)
    for yb in range(YB):
        for kx in range(3):
            for c in range(C):
                for ky in range(3):
                    row = kx * 24 + c * 12 + ky * 4 + yb
                    wblk[row, yb * 32 : (yb + 1) * 32] = wn[:, c, ky, kx]

    f = delay.astype(np.float32)
    coef = np.zeros((128, 6), dtype=np.float32)
    for yb in range(YB):
        s = slice(yb * 32, (yb + 1) * 32)
        coef[s, 0] = f
        coef[s, 1] = 1.0 - f
        coef[s, 2] = -(1.0 - f) / 2.0
        coef[s, 3] = -f / 2.0
        coef[s, 4] = (1.0 - f) / 2.0
        coef[s, 5] = f / 2.0

    # im2col, t-major, pre-scaled by 0.75^-t: xrep[n, t, krow, yg*64+x]
    xpad = np.pad(spike, ((0, 0), (0, 0), (1, 0), (1, 0), (0, 0)))
    xrep = np.empty((N, T, K, Q), dtype=np.float32)
    yg8 = 8 * np.arange(NYG)
    for kx in range(3):
        for ky in range(3):
            for yb in range(4):
                rows = 2 * yb + ky + yg8
                # [n, c, yg, x, t]
                blk = xpad[:, :, rows, kx : kx + 2 * Wp : 2, :]
                for c in range(C):
                    row = kx * 24 + c * 12 + ky * 4 + yb
                    xrep[:, :, row, :] = (
                        blk[:, c].transpose(0, 3, 1, 2).reshape(N, T, Q)
                    )
    upsc = (np.float32(DECAY) ** (-np.arange(T, dtype=np.float32))).astype(np.float32)
    xrep *= upsc[None, :, None, None]
    return xrep, wblk, coef


def _host_post(outs):
    # per-core out [NG, 128, TG*Q] bf16 -> [CH, 64, 64, T] fp32
    full = np.empty((N, CH, Hp, Wp, T), dtype=np.float32)
    for n, o in enumerate(outs):
        a = np.asarray(o).astype(np.float32)
        # [tg, (yb,ch), tt, yg, x] -> [ch, yg, yb, x, tg, tt]
        a = a.reshape(NG, YB, CH, TG, NYG, Wp).transpose(2, 4, 1, 5, 0, 3)
        full[n] = a.reshape(CH, Hp, Wp, T)
    return full


def kernel(spike, weight_v, weight_g, delay):
    global _COMPILED
    if _COMPILED is None:
        _COMPILED = _build_program()
    nc = _COMPILED

    xrep, wblk, coef = _host_prep(spike, weight_v, weight_g, delay)
    in_maps = [
        {"x": np.ascontiguousarray(xrep[n]), "wblk": wblk, "coef": coef}
        for n in range(N)
    ]
    res = bass_utils.run_bass_kernel_spmd(nc, in_maps, core_ids=list(range(N)))
    return _host_post([r["out"] for r in res.results])
